# revision 13
# baseline (speedup 1.0000x reference)
"""3-layer GCN (nn_GCNConvNet) on 8 Trainium2 NeuronCores.

Strategy (dst-partitioned SpMM with replicated feature table):
  - Nodes sharded 8x6250 (padded to 6400 = 50 blocks x 128 slots/core);
    edges partitioned by destination owner.
  - Per layer: every core holds the full fp16 "table" = dis[v] * h[v].
    Layer 0's table is precomputed HOST-side (dis * x) and shipped
    replicated as inputs, so gathers start immediately.  Layer 1/2
    tables are replicated via two AllGathers each (lo half = blocks
    0-24 of every core, hi half = 25-49); the lo AllGather fires
    mid-way through the previous layer, hiding the collective.
  - Aggregation agg[dst] = sum_e dis[src]*h[src] is computed as bulk
    dma_gather of source rows (sorted by dst block) followed by one-hot
    fp8 matmuls accumulating into PSUM per 128-dst block.  One-hots are
    host-precomputed and streamed from DRAM on the HWDGE queue.
    dis[dst] is applied on the PSUM readout.
  - The serial bottleneck is SWDGE descriptor generation on the GpSimd
    Q7 (~2.4-3.4 ns/descriptor, one per gathered edge row): gathers are
    emitted with deep chunk lookahead (lo 10, hi 6 chunks ahead) over
    the 4 SWDGE queues so the Q7 never idles across AllGather waits or
    layer boundaries; collective triggers are placed so their sem waits
    don't head-of-line block gather generation.
  - GCN layer commutes: (A x) W = A (x W), so transform (@W + b, relu)
    runs after aggregation on the own shard only.  PSUM readout and
    dis scaling run on the otherwise-idle ACT engine; bias-add and
    relu*dis run on DVE.
  - dma_gather uses int16 indices (<32768), so the 51200-row table is
    split in two 25600-row tensors (lo/hi); each block's edges are
    grouped lo-first.
"""

import numpy as np
import ml_dtypes

import concourse.bass as bass
import concourse.mybir as mybir
import concourse.tile as tile
from concourse import bacc
from concourse.bass_utils import run_bass_kernel_spmd
from concourse.masks import make_identity

NC = 8
N = 50000
F = 128            # IN_DIM == HID
FOUT = 64
P_OWN = N // NC    # 6250
BLOCKS = 50
HB = BLOCKS // 2       # 25 blocks per half
P_PAD = BLOCKS * 128   # 6400
TAB = NC * P_PAD       # 51200
HALF = TAB // 2        # 25600
import os as _os0
G = int(_os0.environ.get("KG", "2"))   # blocks per gather chunk
NCHUNK = BLOCKS // G                    # gather chunks per layer
CI_LO_READY = -(-HB // G) - 1           # chunk whose compute finishes the lo half

FP16 = mybir.dt.float16

import os as _os
_SKIP = set(_os.environ.get("KSKIP", "").split(",")) - {""}
_SP = _os.environ.get("KSP", "0") == "1"   # single_packet (True crashes HW!)
_KREP = int(_os.environ.get("KREP", "1"))  # pipeline repetitions (timing builds)
_OHGEN = _os.environ.get("KOH", "0") == "1"  # generate one-hots on-chip (DVE)
_MSGB = int(_os.environ.get("KMSGB", "4"))   # msg tiles in flight per tag
_PSB = int(_os.environ.get("KPSB", "4"))     # PSUM bufs per tag


def _wrap_idx(idx: np.ndarray) -> np.ndarray:
    """dma_gather index layout: logical i -> [i%16, i//16], tiled to 128 rows."""
    n = idx.size
    w = idx.reshape(n // 16, 16).T.astype(np.int16)
    return np.tile(w, (8, 1))


def _greedy2d(vs, lo_deg, hi_deg, nb, cap=1024.0):
    """Assign nodes vs to nb blocks (<=128 slots each), jointly balancing
    lo/hi in-degree loads.  Returns (block, slot) per node (aligned to vs)."""
    order = np.argsort(-(lo_deg[vs] + hi_deg[vs]), kind="stable")
    lo_b = np.zeros(nb, np.float64)
    hi_b = np.zeros(nb, np.float64)
    fill = np.zeros(nb, np.int64)
    blk = np.empty(len(vs), np.int64)
    slot = np.empty(len(vs), np.int64)
    for i in order:
        v = vs[i]
        nlo = lo_b + lo_deg[v]
        nhi = hi_b + hi_deg[v]
        cost = nlo**2 + nhi**2 + 1e12 * ((nlo > cap) | (nhi > cap))
        cost[fill >= 128] = np.inf
        b = int(np.argmin(cost))
        blk[i] = b
        slot[i] = fill[b]
        fill[b] += 1
        lo_b[b] += lo_deg[v]
        hi_b[b] += hi_deg[v]
    return blk, slot


def _preprocess(edge_index: np.ndarray):
    """Partition/permute the graph. Returns per-core host arrays + layout.

    Self-loop edges are excluded from the gather lists — the kernel adds
    dis[v]*h[v] per node via an identity matmul on the resident own-table.
    """
    src = np.asarray(edge_index[0], np.int64)
    dst = np.asarray(edge_index[1], np.int64)
    deg = np.bincount(dst, minlength=N) + 1   # + implicit self-loop
    dis = 1.0 / np.sqrt(np.maximum(deg, 1.0))

    # --- pass 1: block assignment with a proxy lo-split (src core < 4)
    lo_mask_p = src < (NC // 2) * P_OWN
    lo_deg1 = np.bincount(dst[lo_mask_p], minlength=N)
    hi_deg1 = np.bincount(dst[~lo_mask_p], minlength=N)
    node_block = np.empty(N, np.int64)   # block within core [0, 50)
    node_slot = np.empty(N, np.int64)
    for c in range(NC):
        vs = np.arange(c * P_OWN, (c + 1) * P_OWN)
        blk, slot = _greedy2d(vs, lo_deg1, hi_deg1, BLOCKS)
        node_block[vs] = blk
        node_slot[vs] = slot

    # --- pass 2: halves fixed by pass 1 (block < HB -> lo); rebalance
    # within each half using the true lo/hi source degrees.
    half = node_block < HB
    lo_deg2 = np.bincount(dst[half[src]], minlength=N)
    hi_deg2 = np.bincount(dst[~half[src]], minlength=N)
    for c in range(NC):
        vs = np.arange(c * P_OWN, (c + 1) * P_OWN)
        for side in (0, 1):
            sel = vs[half[vs] == (side == 0)]
            if len(sel) == 0:
                continue
            blk, slot = _greedy2d(sel, lo_deg2, hi_deg2, HB)
            node_block[sel] = blk + (0 if side == 0 else HB)
            node_slot[sel] = slot
    # halves unchanged by construction

    # table row of node v: lo/hi tensor, core-major, slot-major, block
    core_of = np.arange(N) // P_OWN
    b_loc = node_block % HB
    pp = (
        (node_block >= HB).astype(np.int64) * HALF
        + core_of * (128 * HB)
        + node_slot * HB
        + b_loc
    )

    # --- per-(block, side) edge grouping; lo = src table row < HALF
    e_blk = node_block[dst] + core_of[dst] * BLOCKS    # global dst block
    e_slot = node_slot[dst]
    e_srcpp = pp[src]
    e_lo = e_srcpp < HALF
    key = e_blk * 2 + (~e_lo).astype(np.int64)
    # secondary sort by source row: ascending-address gathers are kinder
    # to HBM row buffers
    order = np.argsort(key * (np.int64(TAB) + 1) + e_srcpp, kind="stable")
    key_s = key[order]
    cnt = np.bincount(key_s, minlength=NC * BLOCKS * 2)
    starts = np.concatenate([[0], np.cumsum(cnt)[:-1]])
    pos = np.arange(len(key_s)) - starts[key_s]

    lo_cnt = cnt[0::2].reshape(NC, BLOCKS)
    hi_cnt = cnt[1::2].reshape(NC, BLOCKS)
    t_lo = int(np.ceil(lo_cnt.max() / 128))
    t_hi = int(np.ceil(hi_cnt.max() / 128))
    t_tot = t_lo + t_hi

    e_srcpp_s = e_srcpp[order]
    e_slot_s = e_slot[order]
    e_lo_s = e_lo[order]
    blk_s = key_s // 2
    core_s = blk_s // BLOCKS
    lb_s = blk_s % BLOCKS

    one = ml_dtypes.float8_e4m3(1.0)
    per_core = []
    for c in range(NC):
        m = core_s == c
        lb = lb_s[m]
        p = pos[m]
        is_lo = e_lo_s[m]
        spp = e_srcpp_s[m]
        slot = e_slot_s[m]

        idx_lo = np.zeros(BLOCKS * t_lo * 128, np.int64)
        sl = is_lo
        idx_lo[lb[sl] * t_lo * 128 + p[sl]] = spp[sl]
        idx_hi = np.zeros(BLOCKS * t_hi * 128, np.int64)
        sh = ~is_lo
        idx_hi[lb[sh] * t_hi * 128 + p[sh]] = spp[sh] - HALF

        # compact one-hot source: dst slot per (row, group), -1 pad
        j = np.where(is_lo, p // 128, t_lo + p // 128)
        g = lb * t_tot + j
        sl8 = np.full((128, BLOCKS * t_tot), -1, np.int8)
        sl8[p % 128, g] = slot.astype(np.int8)
        if _OHGEN:
            oh = None
        else:
            oh = np.zeros((128, BLOCKS * t_tot, 128), ml_dtypes.float8_e4m3)
            oh[p % 128, g, slot] = one

        # wrap indices chunk-wise (each dma_gather gets its own wrapped slab)
        nlo = G * t_lo * 128
        nhi = G * t_hi * 128
        idx_lo_w = np.concatenate(
            [_wrap_idx(idx_lo[ci * nlo:(ci + 1) * nlo]) for ci in range(NCHUNK)],
            axis=1,
        )
        idx_hi_w = np.concatenate(
            [_wrap_idx(idx_hi[ci * nhi:(ci + 1) * nhi]) for ci in range(NCHUNK)],
            axis=1,
        )
        per_core.append(dict(idx_lo=idx_lo_w, idx_hi=idx_hi_w, onehot=oh, slots=sl8))

    return per_core, pp, dis, node_block, node_slot, t_lo, t_hi


def _build_program(t_lo: int, t_hi: int):
    t_tot = t_lo + t_hi
    nc = bacc.Bacc(None, target_bir_lowering=False, num_devices=NC,
                   num_swdge_queues=4)

    tab0_lo_d = nc.dram_tensor("tab0_lo", [HALF, F], FP16, kind="ExternalInput")
    tab0_hi_d = nc.dram_tensor("tab0_hi", [HALF, F], FP16, kind="ExternalInput")
    t0own_d = nc.dram_tensor("t0own", [128, P_PAD], FP16, kind="ExternalInput")
    dis_d = nc.dram_tensor("dis_d", [128, BLOCKS], mybir.dt.float32, kind="ExternalInput")
    idx_lo_d = nc.dram_tensor("idx_lo", [128, BLOCKS * t_lo * 8], mybir.dt.int16, kind="ExternalInput")
    idx_hi_d = nc.dram_tensor("idx_hi", [128, BLOCKS * t_hi * 8], mybir.dt.int16, kind="ExternalInput")
    if _OHGEN:
        slot_d = nc.dram_tensor("slots", [128, BLOCKS * t_tot], mybir.dt.int8, kind="ExternalInput")
        oh_d = None
    else:
        slot_d = None
        oh_d = nc.dram_tensor("onehot", [128, BLOCKS * t_tot, 128], mybir.dt.float8e4, kind="ExternalInput")
    w_d = [
        nc.dram_tensor("w0", [F, F], FP16, kind="ExternalInput"),
        nc.dram_tensor("w1", [F, F], FP16, kind="ExternalInput"),
        nc.dram_tensor("w2", [F, FOUT], FP16, kind="ExternalInput"),
    ]
    bt_d = [
        nc.dram_tensor("bt0", [128, F], mybir.dt.float32, kind="ExternalInput"),
        nc.dram_tensor("bt1", [128, F], mybir.dt.float32, kind="ExternalInput"),
        nc.dram_tensor("bt2", [128, FOUT], mybir.dt.float32, kind="ExternalInput"),
    ]
    out_d = nc.dram_tensor("out", [P_PAD, FOUT], mybir.dt.float32, kind="ExternalOutput")

    with tile.TileContext(nc) as tc:
        with (
            tc.tile_pool(name="const", bufs=1) as cp,
            tc.tile_pool(name="sb", bufs=3) as sb,
            tc.tile_pool(name="xp", bufs=2) as xp,
            tc.tile_pool(name="tabp", bufs=2) as tabp,
            tc.tile_pool(name="msgp", bufs=_MSGB) as msgp,
            tc.tile_pool(name="ohp", bufs=3) as ohp,
            tc.tile_pool(name="ps", bufs=_PSB, space="PSUM") as ps,
            tc.tile_pool(name="dr", bufs=1, space="DRAM") as dr,
        ):
            # ---- constants (gather indexes first: first gather gen
            # depends only on these)
            il_sb = cp.tile([128, BLOCKS * t_lo * 8], mybir.dt.int16)
            nc.sync.dma_start(il_sb[:], idx_lo_d[:])
            ih_sb = cp.tile([128, BLOCKS * t_hi * 8], mybir.dt.int16)
            nc.sync.dma_start(ih_sb[:], idx_hi_d[:])
            w_sb, bt_sb = [], []
            for l in range(3):
                fo = F if l < 2 else FOUT
                wt = cp.tile([F, fo], FP16, name=f"w{l}_sb")
                nc.sync.dma_start(wt[:], w_d[l][:])
                bt = cp.tile([128, fo], mybir.dt.float32, name=f"bt{l}_sb")
                nc.sync.dma_start(bt[:], bt_d[l][:])
                w_sb.append(wt)
                bt_sb.append(bt)
            dis_sb = cp.tile([128, BLOCKS], mybir.dt.float32)
            nc.sync.dma_start(dis_sb[:], dis_d[:])
            ident16 = cp.tile([128, 128], FP16)
            make_identity(nc, ident16[:])
            if _OHGEN:
                slot_sb = cp.tile([128, BLOCKS * t_tot], mybir.dt.int8)
                nc.sync.dma_start(slot_sb[:], slot_d[:])
                iota_sb = cp.tile([128, G * t_tot * 128], mybir.dt.int8)
                nc.gpsimd.iota(
                    iota_sb[:], pattern=[[0, G * t_tot], [1, 128]], base=0,
                    channel_multiplier=0, allow_small_or_imprecise_dtypes=True,
                )
            else:
                slot_sb = iota_sb = None

            # ---- DRAM scratch: AllGather bounce + replicated tables
            # (distinct sets per rep: Shared tensors allow only one writer)
            nsets = _KREP
            sets = []
            for s in range(nsets):
                sfx = f"_s{s}" if nsets > 1 else ""
                ags = [None]   # [layer][side] bounce buffers (layer 0 is input)
                tabs = [(tab0_lo_d, tab0_hi_d)]
                for l in range(1, 3):
                    a_lo = dr.tile([128, HB * 128], FP16, name=f"ag_lo{l}{sfx}")
                    a_hi = dr.tile([128, HB * 128], FP16, name=f"ag_hi{l}{sfx}")
                    t_lo_t = dr.tile([HALF, F], FP16, addr_space="Shared", name=f"tab_lo{l}{sfx}")
                    t_hi_t = dr.tile([HALF, F], FP16, addr_space="Shared", name=f"tab_hi{l}{sfx}")
                    ags.append((a_lo, a_hi))
                    tabs.append((t_lo_t, t_hi_t))
                sets.append((ags, tabs))

            gather_k = 0
            for rep in range(_KREP):
                ags, tabs = sets[rep % nsets]
                _emit_pipeline(
                    nc, ags, tabs, t0own_d, dis_sb, il_sb, ih_sb, ident16,
                    w_sb, bt_sb, (oh_d, slot_sb, iota_sb), out_d,
                    xp, tabp, msgp, ohp, ps, sb, t_lo, t_hi, gather_k,
                )
                gather_k += 6 * NCHUNK

    nc.compile()
    return nc


def _emit_pipeline(nc, ags, tabs, t0own_d, dis_sb, il_sb, ih_sb, ident16,
                   w_sb, bt_sb, ohsrc, out_d, xp, tabp, msgp, ohp,
                   ps, sb, t_lo, t_hi, gather_k):
    oh_d, slot_sb, iota_sb = ohsrc
    t_tot = t_lo + t_hi
    nlo = G * t_lo * 128
    nhi = G * t_hi * 128
    GC = G * 128   # table cols per chunk
    LA = max(2, 20 // G)   # lo-gather lookahead (chunks)
    LH = max(2, 12 // G)   # hi-gather lookahead (chunks)

    gk = [gather_k]
    msgs = {}      # (side, ci) -> msg tile, for the current layer

    def emit_ag(l, side):
        """Trigger the AllGather of (layer l, side) into its tab tensor."""
        if "ag" in _SKIP:
            return
        nc.gpsimd.collective_compute(
            "AllGather", mybir.AluOpType.bypass,
            replica_groups=[list(range(NC))],
            ins=[ags[l][side].opt()],
            outs=[tabs[l][side].opt()],
        )

    def emit_table_chunk_out(l, ci, table_tile):
        """DMA chunk ci of layer l's table into its AG bounce buffer(s)."""
        b0, b1 = ci * G, (ci + 1) * G
        for side, lo_b, hi_b in ((0, b0, min(b1, HB)), (1, max(b0, HB), b1)):
            if lo_b >= hi_b:
                continue
            cl = lo_b - (0 if side == 0 else HB)
            n = hi_b - lo_b
            nc.sync.dma_start(
                ags[l][side][:, cl * 128:(cl + n) * 128],
                table_tile[:, lo_b * 128:hi_b * 128],
            )

    def emit_gather(l, ci, side):
        if side == 0:
            msg = msgp.tile([128, G * t_lo, F], FP16, tag="mlo", bufs=LA + 2)
            if "gather" not in _SKIP:
                nc.gpsimd.dma_gather(
                    msg[:], tabs[l][0][:],
                    il_sb[:, ci * G * t_lo * 8:(ci + 1) * G * t_lo * 8],
                    nlo, nlo, F, single_packet=_SP,
                    queue_num=gk[0] % 4,
                )
        else:
            msg = msgp.tile([128, G * t_hi, F], FP16, tag="mhi", bufs=LH + 2)
            if "gather" not in _SKIP:
                nc.gpsimd.dma_gather(
                    msg[:], tabs[l][1][:],
                    ih_sb[:, ci * G * t_hi * 8:(ci + 1) * G * t_hi * 8],
                    nhi, nhi, F, single_packet=_SP,
                    queue_num=gk[0] % 4,
                )
        gk[0] += 1
        msgs[(side, ci)] = msg

    # ---- layer-0 table is precomputed host-side (dis * x, replicated
    # in DRAM as tab0_lo/tab0_hi inputs); just load the own shard for
    # the self-loop matmuls and start gathering immediately.
    table0 = tabp.tile([128, P_PAD], FP16, tag="table")
    nc.sync.dma_start(table0[:], t0own_d[:])
    for k in range(LA):
        emit_gather(0, k, 0)
    for k in range(LH):
        emit_gather(0, k, 1)

    # ---- 3 GCN layers
    table_cur = table0
    for l in range(3):
        fo = F if l < 2 else FOUT
        table_next = (
            tabp.tile([128, P_PAD], FP16, tag="table", name=f"table{l+1}")
            if l < 2 else None
        )
        for ci in range(NCHUNK):
            if ci + LA < NCHUNK:
                emit_gather(l, ci + LA, 0)
            if ci + LH < NCHUNK:
                emit_gather(l, ci + LH, 1)
            msg_lo = msgs.pop((0, ci))
            msg_hi = msgs.pop((1, ci))
            oh = ohp.tile([128, G * t_tot, 128], mybir.dt.float8e4, tag="oh")
            if "oh" not in _SKIP:
                if _OHGEN:
                    sl = slot_sb[:, ci * G * t_tot:(ci + 1) * G * t_tot]
                    sl_b = bass.AP(
                        sl.tensor, sl.offset,
                        list(sl.ap[:-1]) + [list(sl.ap[-1]), [0, 128]],
                    )
                    nc.vector.tensor_tensor(
                        out=oh[:], in0=iota_sb[:], in1=sl_b,
                        op=mybir.AluOpType.is_equal,
                    )
                else:
                    nc.scalar.dma_start(
                        oh[:], oh_d[:, ci * G * t_tot:(ci + 1) * G * t_tot, :]
                    )
            for bi in range(G):
                b = ci * G + bi
                agg_ps = ps.tile([128, 128], mybir.dt.float32, tag="agg", space="PSUM")
                # transposed segsum: aggT[feat, dst] += msg.T @ onehot
                if "mm" not in _SKIP:
                    for j in range(t_tot):
                        lhs = (
                            msg_lo[:, bi * t_lo + j, :] if j < t_lo
                            else msg_hi[:, bi * t_hi + (j - t_lo), :]
                        )
                        nc.tensor.matmul(
                            agg_ps[:], lhsT=lhs, rhs=oh[:, bi * t_tot + j, :],
                            start=(j == 0), stop=False,
                        )
                # self-loop: += table_blk.T @ I
                nc.tensor.matmul(
                    agg_ps[:], lhsT=table_cur[:, b * 128:(b + 1) * 128],
                    rhs=ident16[:],
                    start=("mm" in _SKIP), stop=True,
                )
                # PSUM readout on the ACT engine (keeps DVE free)
                aggT = sb.tile([128, 128], FP16, tag="aggT")
                nc.scalar.copy(aggT[:], agg_ps[:])
                h_ps = ps.tile([128, F], mybir.dt.float32, tag="hps", space="PSUM")
                nc.tensor.matmul(
                    h_ps[:, :fo], lhsT=aggT[:], rhs=w_sb[l][:],
                    start=True, stop=True,
                )
                # t1 = dis * h  (ACT reads PSUM, frees the bank early)
                t1 = sb.tile([128, F], mybir.dt.float32, tag="t1")
                nc.scalar.mul(t1[:, :fo], h_ps[:, :fo], dis_sb[:, b:b + 1])
                if l < 2:
                    # tmp2 = t1 + bias; table_next = dis * relu(tmp2)
                    tmp2 = sb.tile([128, F], mybir.dt.float32, tag="tmp2")
                    nc.vector.tensor_tensor(
                        out=tmp2[:], in0=t1[:], in1=bt_sb[l][:],
                        op=mybir.AluOpType.add,
                    )
                    nc.vector.tensor_scalar(
                        out=table_next[:, b * 128:(b + 1) * 128],
                        in0=tmp2[:],
                        scalar1=0.0, scalar2=dis_sb[:, b:b + 1],
                        op0=mybir.AluOpType.max, op1=mybir.AluOpType.mult,
                    )
                else:
                    ob = sb.tile([128, FOUT], mybir.dt.float32, tag="ob")
                    nc.vector.tensor_tensor(
                        out=ob[:], in0=t1[:, :FOUT], in1=bt_sb[2][:],
                        op=mybir.AluOpType.add,
                    )
                    nc.sync.dma_start(out_d[b * 128:(b + 1) * 128, :], ob[:])
            if l < 2:
                emit_table_chunk_out(l + 1, ci, table_next)
                # lo AG for the next layer: data ready after chunk HCHUNK-1;
                # trigger emitted 2 chunks later so queued gathers are not
                # head-of-line blocked behind its wait.
                if ci == CI_LO_READY + 5:
                    emit_ag(l + 1, 0)
        if l < 2:
            # next-layer prologue: lo gathers (tab_lo already gathered),
            # then the hi AG trigger, then the leading hi gathers.
            for k in range(LA):
                emit_gather(l + 1, k, 0)
            emit_ag(l + 1, 1)
            for k in range(LH):
                emit_gather(l + 1, k, 1)
            table_cur = table_next


def _timed_run(nc, in_maps, iters=5):
    """Mirror run_bass_via_pjrt's multi-core path, but keep inputs device-
    resident and time repeated executions (returns results, best_ns)."""
    import time
    import jax

    sharded, concat_in, make_zeros, unpack = _make_sharded(nc, in_maps)
    # Differential pipelined timing: dispatch n asynchronously, block once.
    # per-exec = (T(n_long) - T(n_short)) / (n_long - n_short) cancels the
    # RPC floor. Repeat pairs and take the min slope.
    n_short, n_long, pairs = 4, 16, max(2, iters)
    n_total = 1 + pairs * (n_short + n_long)
    zero_sets = [make_zeros() for _ in range(n_total)]
    out_arrs = jax.block_until_ready(sharded(*concat_in, *zero_sets[0]))

    def pipe(k0, n):
        t0 = time.perf_counter()
        rs = [sharded(*concat_in, *zero_sets[k0 + k]) for k in range(n)]
        last = jax.block_until_ready(rs[-1])
        dt = time.perf_counter() - t0
        for r in rs[:-1]:
            del r
        return dt, last

    times = []
    slopes = []
    k0 = 1
    for _ in range(pairs):
        t_s, _ = pipe(k0, n_short)
        k0 += n_short
        t_l, out_arrs = pipe(k0, n_long)
        k0 += n_long
        slopes.append((t_l - t_s) / (n_long - n_short))
        times.extend([t_s, t_l])
    pos = [s for s in slopes if s > 0] or slopes
    per_exec = min(pos)
    best_ns = int(per_exec * 1e9)
    kernel._last_slopes = [int(s * 1e9) for s in slopes]
    results = unpack(out_arrs)
    return results, best_ns, times


def _make_sharded(nc, in_maps):
    """Build the sharded jit fn + device-resident inputs (shared helper)."""
    import jax
    from jax.sharding import Mesh, PartitionSpec, NamedSharding
    from jax.experimental.shard_map import shard_map
    import concourse.mybir as mb
    from concourse.bass2jax import (
        _bass_exec_p, partition_id_tensor, install_neuronx_cc_hook,
    )

    install_neuronx_cc_hook()
    n_cores = len(in_maps)
    partition_name = nc.partition_id_tensor.name if nc.partition_id_tensor else None
    in_names, out_names, out_avals, zero_outs = [], [], [], []
    for alloc in nc.m.functions[0].allocations:
        if not isinstance(alloc, mb.MemoryLocationSet):
            continue
        name = alloc.memorylocations[0].name
        if alloc.kind == "ExternalInput":
            if name != partition_name:
                in_names.append(name)
        elif alloc.kind == "ExternalOutput":
            out_names.append(name)
            shape = tuple(alloc.tensor_shape)
            dtype = mb.dt.np(alloc.dtype)
            out_avals.append(jax.core.ShapedArray(shape, dtype))
            zero_outs.append(np.zeros(shape, dtype))
    n_params = len(in_names)
    n_outs = len(out_avals)
    in_names.extend(out_names)
    if partition_name is not None:
        in_names.append(partition_name)
    donate = tuple(range(n_params, n_params + n_outs))

    def _body(*args):
        operands = list(args)
        if partition_name is not None:
            operands.append(partition_id_tensor())
        return tuple(_bass_exec_p.bind(
            *operands,
            out_avals=tuple(out_avals), in_names=tuple(in_names),
            out_names=tuple(out_names), lowering_input_output_aliases=(),
            sim_require_finite=True, sim_require_nnan=True, nc=nc,
        ))

    devices = jax.devices()[:n_cores]
    mesh = Mesh(np.asarray(devices), ("core",))
    spec = NamedSharding(mesh, PartitionSpec("core"))
    sharded = jax.jit(
        shard_map(_body, mesh=mesh,
                  in_specs=(PartitionSpec("core"),) * (n_params + n_outs),
                  out_specs=(PartitionSpec("core"),) * n_outs,
                  check_rep=False),
        donate_argnums=donate, keep_unused=True,
    )
    concat_in = [
        jax.device_put(
            np.concatenate([np.asarray(in_maps[c][in_names[i]]) for c in range(n_cores)], axis=0),
            spec,
        )
        for i in range(n_params)
    ]

    def make_zeros():
        return [jax.device_put(np.zeros((n_cores * z.shape[0], *z.shape[1:]), z.dtype), spec)
                for z in zero_outs]

    def unpack(out_arrs):
        return [
            {name: np.asarray(out_arrs[i]).reshape(n_cores, *out_avals[i].shape)[c]
             for i, name in enumerate(out_names)}
            for c in range(n_cores)
        ]

    return sharded, concat_in, make_zeros, unpack


def _prepare(x, edge_index, W0, b0, W1, b1, W2, b2):
    """Preprocess + build program + per-core input maps.

    Returns (nc, in_maps, unshard) where unshard(results) -> full output.
    """
    x = np.asarray(x)
    edge_index = np.asarray(edge_index)
    per_core, pp, dis, node_block, node_slot, t_lo, t_hi = _preprocess(edge_index)

    nc = _build_program(t_lo, t_hi)

    w0 = np.ascontiguousarray(np.asarray(W0, np.float16))
    w1 = np.ascontiguousarray(np.asarray(W1, np.float16))
    w2 = np.ascontiguousarray(np.asarray(W2, np.float16))
    bt0 = np.tile(np.asarray(b0, np.float32)[None, :], (128, 1))
    bt1 = np.tile(np.asarray(b1, np.float32)[None, :], (128, 1))
    bt2 = np.tile(np.asarray(b2, np.float32)[None, :], (128, 1))

    # host-precomputed layer-0 table (dis * x), in the AllGather layout
    pre0 = (dis[:, None] * np.asarray(x, np.float32)).astype(np.float16)
    tab0_lo = np.zeros((HALF, F), np.float16)
    tab0_hi = np.zeros((HALF, F), np.float16)
    lo_m = pp < HALF
    tab0_lo[pp[lo_m]] = pre0[lo_m]
    tab0_hi[pp[~lo_m] - HALF] = pre0[~lo_m]
    tab0_lo = np.ascontiguousarray(tab0_lo)
    tab0_hi = np.ascontiguousarray(tab0_hi)

    in_maps = []
    for c in range(NC):
        vs = np.arange(c * P_OWN, (c + 1) * P_OWN)
        rows = node_block[vs] * 128 + node_slot[vs]  # padded local row
        t_nm = np.zeros((P_PAD, F), np.float16)
        t_nm[rows] = pre0[vs]
        # p-major: [128 slot, BLOCKS*128] where col b*128+f = tab[node(b,p), f]
        t0own = np.ascontiguousarray(
            t_nm.reshape(BLOCKS, 128, F).transpose(1, 0, 2).reshape(128, P_PAD)
        )
        dis_b = np.zeros((128, BLOCKS), np.float32)
        dis_b[node_slot[vs], node_block[vs]] = dis[vs]
        d = per_core[c]
        im = dict(
            tab0_lo=tab0_lo, tab0_hi=tab0_hi, t0own=t0own, dis_d=dis_b,
            idx_lo=np.ascontiguousarray(d["idx_lo"]),
            idx_hi=np.ascontiguousarray(d["idx_hi"]),
            w0=w0, w1=w1, w2=w2, bt0=bt0, bt1=bt1, bt2=bt2,
        )
        if _OHGEN:
            im["slots"] = np.ascontiguousarray(d["slots"])
        else:
            im["onehot"] = np.ascontiguousarray(d["onehot"])
        in_maps.append(im)

    def unshard(results):
        out = np.empty((N, FOUT), np.float32)
        for c in range(NC):
            vs = np.arange(c * P_OWN, (c + 1) * P_OWN)
            rows = node_block[vs] * 128 + node_slot[vs]
            out[vs] = results[c]["out"][rows]
        return out

    return nc, in_maps, unshard


def kernel(x, edge_index, W0, b0, W1, b1, W2, b2, _trace=False, _bench_iters=0):
    nc, in_maps, unshard = _prepare(x, edge_index, W0, b0, W1, b1, W2, b2)

    if _bench_iters:
        results, best_ns, times = _timed_run(nc, in_maps, iters=_bench_iters)
        kernel._last_time_ns = best_ns
        kernel._last_times = times
    else:
        res = run_bass_kernel_spmd(nc, in_maps, core_ids=list(range(NC)), trace=_trace)
        results = res.results
        if _trace:
            kernel._last_result = res

    return unshard(results)


# revision 14
# speedup vs baseline: 1.1693x; 1.1693x over previous
"""3-layer GCN (nn_GCNConvNet) on 8 Trainium2 NeuronCores.

Strategy (dst-partitioned SpMM with replicated feature table):
  - Nodes sharded 8x6250 (padded to 6400 = 50 blocks x 128 slots/core);
    edges partitioned by destination owner.
  - Per layer: every core holds the full fp16 "table" = dis[v] * h[v].
    Layer 0's table is precomputed HOST-side (dis * x) and shipped
    replicated as inputs, so gathers start immediately.  Layer 1/2
    tables are replicated via two AllGathers each (lo half = blocks
    0-24 of every core, hi half = 25-49); the lo AllGather fires
    mid-way through the previous layer, hiding the collective.
  - Aggregation agg[dst] = sum_e dis[src]*h[src] is computed as bulk
    dma_gather of source rows (sorted by dst block) followed by one-hot
    fp8 matmuls accumulating into PSUM per 128-dst block.  One-hots are
    host-precomputed and streamed from DRAM on the HWDGE queue.
    dis[dst] is applied on the PSUM readout.
  - The serial bottleneck is SWDGE descriptor generation on the GpSimd
    Q7 (~2.4-3.4 ns/descriptor, one per gathered edge row): gathers are
    emitted with deep chunk lookahead (lo 10, hi 6 chunks ahead) over
    the 4 SWDGE queues so the Q7 never idles across AllGather waits or
    layer boundaries; collective triggers are placed so their sem waits
    don't head-of-line block gather generation.
  - GCN layer commutes: (A x) W = A (x W), so transform (@W + b, relu)
    runs after aggregation on the own shard only.  PSUM readout and
    dis scaling run on the otherwise-idle ACT engine; bias-add and
    relu*dis run on DVE.
  - dma_gather uses int16 indices (<32768), so the 51200-row table is
    split in two 25600-row tensors (lo/hi); each block's edges are
    grouped lo-first.
"""

import numpy as np
import ml_dtypes

import concourse.bass as bass
import concourse.mybir as mybir
import concourse.tile as tile
from concourse import bacc
from concourse.bass_utils import run_bass_kernel_spmd
from concourse.masks import make_identity

NC = 8
N = 50000
F = 128            # IN_DIM == HID
FOUT = 64
P_OWN = N // NC    # 6250
BLOCKS = 50
HB = BLOCKS // 2       # 25 blocks per half
P_PAD = BLOCKS * 128   # 6400
TAB = NC * P_PAD       # 51200
HALF = TAB // 2        # 25600
import os as _os0
G = int(_os0.environ.get("KG", "2"))   # blocks per gather chunk
NCHUNK = BLOCKS // G                    # gather chunks per layer
CI_LO_READY = -(-HB // G) - 1           # chunk whose compute finishes the lo half

FP16 = mybir.dt.float16

import os as _os
_SKIP = set(_os.environ.get("KSKIP", "").split(",")) - {""}
_SP = _os.environ.get("KSP", "0") == "1"   # single_packet (True crashes HW!)
_KREP = int(_os.environ.get("KREP", "1"))  # pipeline repetitions (timing builds)
_OHGEN = _os.environ.get("KOH", "0") == "1"  # generate one-hots on-chip (DVE)
_MSGB = int(_os.environ.get("KMSGB", "4"))   # msg tiles in flight per tag
_PSB = int(_os.environ.get("KPSB", "4"))     # PSUM bufs per tag


def _wrap_idx(idx: np.ndarray) -> np.ndarray:
    """dma_gather index layout: logical i -> [i%16, i//16], tiled to 128 rows."""
    n = idx.size
    w = idx.reshape(n // 16, 16).T.astype(np.int16)
    return np.tile(w, (8, 1))


def _greedy2d(vs, lo_deg, hi_deg, nb, cap=1024.0):
    """Assign nodes vs to nb blocks (<=128 slots each), jointly balancing
    lo/hi in-degree loads.  Returns (block, slot) per node (aligned to vs)."""
    order = np.argsort(-(lo_deg[vs] + hi_deg[vs]), kind="stable")
    lo_b = np.zeros(nb, np.float64)
    hi_b = np.zeros(nb, np.float64)
    fill = np.zeros(nb, np.int64)
    blk = np.empty(len(vs), np.int64)
    slot = np.empty(len(vs), np.int64)
    for i in order:
        v = vs[i]
        nlo = lo_b + lo_deg[v]
        nhi = hi_b + hi_deg[v]
        cost = nlo**2 + nhi**2 + 1e12 * ((nlo > cap) | (nhi > cap))
        cost[fill >= 128] = np.inf
        b = int(np.argmin(cost))
        blk[i] = b
        slot[i] = fill[b]
        fill[b] += 1
        lo_b[b] += lo_deg[v]
        hi_b[b] += hi_deg[v]
    return blk, slot


def _preprocess(edge_index: np.ndarray):
    """Partition/permute the graph. Returns per-core host arrays + layout.

    Self-loop edges are excluded from the gather lists — the kernel adds
    dis[v]*h[v] per node via an identity matmul on the resident own-table.
    """
    src = np.asarray(edge_index[0], np.int64)
    dst = np.asarray(edge_index[1], np.int64)
    deg = np.bincount(dst, minlength=N) + 1   # + implicit self-loop
    dis = 1.0 / np.sqrt(np.maximum(deg, 1.0))

    # --- pass 1: block assignment with a proxy lo-split (src core < 4)
    lo_mask_p = src < (NC // 2) * P_OWN
    lo_deg1 = np.bincount(dst[lo_mask_p], minlength=N)
    hi_deg1 = np.bincount(dst[~lo_mask_p], minlength=N)
    node_block = np.empty(N, np.int64)   # block within core [0, 50)
    node_slot = np.empty(N, np.int64)
    for c in range(NC):
        vs = np.arange(c * P_OWN, (c + 1) * P_OWN)
        blk, slot = _greedy2d(vs, lo_deg1, hi_deg1, BLOCKS)
        node_block[vs] = blk
        node_slot[vs] = slot

    # --- pass 2: halves fixed by pass 1 (block < HB -> lo); rebalance
    # within each half using the true lo/hi source degrees.
    half = node_block < HB
    lo_deg2 = np.bincount(dst[half[src]], minlength=N)
    hi_deg2 = np.bincount(dst[~half[src]], minlength=N)
    for c in range(NC):
        vs = np.arange(c * P_OWN, (c + 1) * P_OWN)
        for side in (0, 1):
            sel = vs[half[vs] == (side == 0)]
            if len(sel) == 0:
                continue
            blk, slot = _greedy2d(sel, lo_deg2, hi_deg2, HB)
            node_block[sel] = blk + (0 if side == 0 else HB)
            node_slot[sel] = slot
    # halves unchanged by construction

    # table row of node v: lo/hi tensor, core-major, slot-major, block
    core_of = np.arange(N) // P_OWN
    b_loc = node_block % HB
    pp = (
        (node_block >= HB).astype(np.int64) * HALF
        + core_of * (128 * HB)
        + node_slot * HB
        + b_loc
    )

    # --- per-(block, side) edge grouping; lo = src table row < HALF
    e_blk = node_block[dst] + core_of[dst] * BLOCKS    # global dst block
    e_slot = node_slot[dst]
    e_srcpp = pp[src]
    e_lo = e_srcpp < HALF
    key = e_blk * 2 + (~e_lo).astype(np.int64)
    # secondary sort by source row: ascending-address gathers are kinder
    # to HBM row buffers
    order = np.argsort(key * (np.int64(TAB) + 1) + e_srcpp, kind="stable")
    key_s = key[order]
    cnt = np.bincount(key_s, minlength=NC * BLOCKS * 2)
    starts = np.concatenate([[0], np.cumsum(cnt)[:-1]])
    pos = np.arange(len(key_s)) - starts[key_s]

    lo_cnt = cnt[0::2].reshape(NC, BLOCKS)
    hi_cnt = cnt[1::2].reshape(NC, BLOCKS)
    t_lo = int(np.ceil(lo_cnt.max() / 128))
    t_hi = int(np.ceil(hi_cnt.max() / 128))
    t_tot = t_lo + t_hi

    e_srcpp_s = e_srcpp[order]
    e_slot_s = e_slot[order]
    e_lo_s = e_lo[order]
    blk_s = key_s // 2
    core_s = blk_s // BLOCKS
    lb_s = blk_s % BLOCKS

    one = ml_dtypes.float8_e4m3(1.0)
    per_core = []
    for c in range(NC):
        m = core_s == c
        lb = lb_s[m]
        p = pos[m]
        is_lo = e_lo_s[m]
        spp = e_srcpp_s[m]
        slot = e_slot_s[m]

        idx_lo = np.zeros(BLOCKS * t_lo * 128, np.int64)
        sl = is_lo
        idx_lo[lb[sl] * t_lo * 128 + p[sl]] = spp[sl]
        idx_hi = np.zeros(BLOCKS * t_hi * 128, np.int64)
        sh = ~is_lo
        idx_hi[lb[sh] * t_hi * 128 + p[sh]] = spp[sh] - HALF

        # compact one-hot source: dst slot per (row, group), -1 pad
        j = np.where(is_lo, p // 128, t_lo + p // 128)
        g = lb * t_tot + j
        sl8 = np.full((128, BLOCKS * t_tot), -1, np.int8)
        sl8[p % 128, g] = slot.astype(np.int8)
        if _OHGEN:
            oh = None
        else:
            oh = np.zeros((128, BLOCKS * t_tot, 128), ml_dtypes.float8_e4m3)
            oh[p % 128, g, slot] = one

        # wrap indices chunk-wise (each dma_gather gets its own wrapped slab)
        nlo = G * t_lo * 128
        nhi = G * t_hi * 128
        idx_lo_w = np.concatenate(
            [_wrap_idx(idx_lo[ci * nlo:(ci + 1) * nlo]) for ci in range(NCHUNK)],
            axis=1,
        )
        idx_hi_w = np.concatenate(
            [_wrap_idx(idx_hi[ci * nhi:(ci + 1) * nhi]) for ci in range(NCHUNK)],
            axis=1,
        )
        per_core.append(dict(idx_lo=idx_lo_w, idx_hi=idx_hi_w, onehot=oh, slots=sl8))

    return per_core, pp, dis, node_block, node_slot, t_lo, t_hi


def _build_program(t_lo: int, t_hi: int):
    t_tot = t_lo + t_hi
    nc = bacc.Bacc(None, target_bir_lowering=False, num_devices=NC,
                   num_swdge_queues=4)

    tab0_lo_d = nc.dram_tensor("tab0_lo", [HALF, F], FP16, kind="ExternalInput")
    tab0_hi_d = nc.dram_tensor("tab0_hi", [HALF, F], FP16, kind="ExternalInput")
    t0own_d = nc.dram_tensor("t0own", [128, P_PAD], FP16, kind="ExternalInput")
    dis_d = nc.dram_tensor("dis_d", [128, BLOCKS], mybir.dt.float32, kind="ExternalInput")
    idx_lo_d = nc.dram_tensor("idx_lo", [128, BLOCKS * t_lo * 8], mybir.dt.int16, kind="ExternalInput")
    idx_hi_d = nc.dram_tensor("idx_hi", [128, BLOCKS * t_hi * 8], mybir.dt.int16, kind="ExternalInput")
    if _OHGEN:
        slot_d = nc.dram_tensor("slots", [128, BLOCKS * t_tot], mybir.dt.int8, kind="ExternalInput")
        oh_d = None
    else:
        slot_d = None
        oh_d = nc.dram_tensor("onehot", [128, BLOCKS * t_tot, 128], mybir.dt.float8e4, kind="ExternalInput")
    w_d = [
        nc.dram_tensor("w0", [F, F], FP16, kind="ExternalInput"),
        nc.dram_tensor("w1", [F, F], FP16, kind="ExternalInput"),
        nc.dram_tensor("w2", [F, FOUT], FP16, kind="ExternalInput"),
    ]
    bt_d = [
        nc.dram_tensor("bt0", [128, F], mybir.dt.float32, kind="ExternalInput"),
        nc.dram_tensor("bt1", [128, F], mybir.dt.float32, kind="ExternalInput"),
        nc.dram_tensor("bt2", [128, FOUT], mybir.dt.float32, kind="ExternalInput"),
    ]
    out_d = nc.dram_tensor("out", [P_PAD, FOUT], mybir.dt.float32, kind="ExternalOutput")

    with tile.TileContext(nc) as tc:
        with (
            tc.tile_pool(name="const", bufs=1) as cp,
            tc.tile_pool(name="sb", bufs=3) as sb,
            tc.tile_pool(name="xp", bufs=2) as xp,
            tc.tile_pool(name="tabp", bufs=2) as tabp,
            tc.tile_pool(name="msgp", bufs=_MSGB) as msgp,
            tc.tile_pool(name="ohp", bufs=3) as ohp,
            tc.tile_pool(name="ps", bufs=_PSB, space="PSUM") as ps,
            tc.tile_pool(name="dr", bufs=1, space="DRAM") as dr,
        ):
            # ---- constants (gather indexes first: first gather gen
            # depends only on these; first chunks' slabs land first)
            il_sb = cp.tile([128, BLOCKS * t_lo * 8], mybir.dt.int16)
            sl0 = 4 * G * t_lo * 8
            nc.sync.dma_start(il_sb[:, :sl0], idx_lo_d[:, :sl0])
            nc.sync.dma_start(il_sb[:, sl0:], idx_lo_d[:, sl0:])
            ih_sb = cp.tile([128, BLOCKS * t_hi * 8], mybir.dt.int16)
            sh0 = 4 * G * t_hi * 8
            nc.sync.dma_start(ih_sb[:, :sh0], idx_hi_d[:, :sh0])
            nc.sync.dma_start(ih_sb[:, sh0:], idx_hi_d[:, sh0:])
            w_sb, bt_sb = [], []
            for l in range(3):
                fo = F if l < 2 else FOUT
                wt = cp.tile([F, fo], FP16, name=f"w{l}_sb")
                nc.sync.dma_start(wt[:], w_d[l][:])
                bt = cp.tile([128, fo], mybir.dt.float32, name=f"bt{l}_sb")
                nc.sync.dma_start(bt[:], bt_d[l][:])
                w_sb.append(wt)
                bt_sb.append(bt)
            dis_sb = cp.tile([128, BLOCKS], mybir.dt.float32)
            nc.sync.dma_start(dis_sb[:], dis_d[:])
            ident16 = cp.tile([128, 128], FP16)
            make_identity(nc, ident16[:])
            if _OHGEN:
                slot_sb = cp.tile([128, BLOCKS * t_tot], mybir.dt.int8)
                nc.sync.dma_start(slot_sb[:], slot_d[:])
                iota_sb = cp.tile([128, G * t_tot * 128], mybir.dt.int8)
                nc.gpsimd.iota(
                    iota_sb[:], pattern=[[0, G * t_tot], [1, 128]], base=0,
                    channel_multiplier=0, allow_small_or_imprecise_dtypes=True,
                )
            else:
                slot_sb = iota_sb = None

            # ---- DRAM scratch: AllGather bounce + replicated tables
            # (distinct sets per rep: Shared tensors allow only one writer)
            nsets = _KREP
            sets = []
            for s in range(nsets):
                sfx = f"_s{s}" if nsets > 1 else ""
                ags = [None]   # [layer][side] bounce buffers (layer 0 is input)
                tabs = [(tab0_lo_d, tab0_hi_d)]
                for l in range(1, 3):
                    a_lo = dr.tile([128, HB * 128], FP16, name=f"ag_lo{l}{sfx}")
                    a_hi = dr.tile([128, HB * 128], FP16, name=f"ag_hi{l}{sfx}")
                    t_lo_t = dr.tile([HALF, F], FP16, addr_space="Shared", name=f"tab_lo{l}{sfx}")
                    t_hi_t = dr.tile([HALF, F], FP16, addr_space="Shared", name=f"tab_hi{l}{sfx}")
                    ags.append((a_lo, a_hi))
                    tabs.append((t_lo_t, t_hi_t))
                sets.append((ags, tabs))

            gather_k = 0
            for rep in range(_KREP):
                ags, tabs = sets[rep % nsets]
                _emit_pipeline(
                    nc, ags, tabs, t0own_d, dis_sb, il_sb, ih_sb, ident16,
                    w_sb, bt_sb, (oh_d, slot_sb, iota_sb), out_d,
                    xp, tabp, msgp, ohp, ps, sb, t_lo, t_hi, gather_k,
                )
                gather_k += 6 * NCHUNK

    nc.compile()
    return nc


def _emit_pipeline(nc, ags, tabs, t0own_d, dis_sb, il_sb, ih_sb, ident16,
                   w_sb, bt_sb, ohsrc, out_d, xp, tabp, msgp, ohp,
                   ps, sb, t_lo, t_hi, gather_k):
    oh_d, slot_sb, iota_sb = ohsrc
    t_tot = t_lo + t_hi
    nlo = G * t_lo * 128
    nhi = G * t_hi * 128
    GC = G * 128   # table cols per chunk
    LA = max(2, 20 // G)   # lo-gather lookahead (chunks)
    LH = max(2, 12 // G)   # hi-gather lookahead (chunks)

    gk = [gather_k]
    msgs = {}      # (side, ci) -> msg tile, for the current layer

    def emit_ag(l, side):
        """Trigger the AllGather of (layer l, side) into its tab tensor."""
        if "ag" in _SKIP:
            return
        nc.gpsimd.collective_compute(
            "AllGather", mybir.AluOpType.bypass,
            replica_groups=[list(range(NC))],
            ins=[ags[l][side].opt()],
            outs=[tabs[l][side].opt()],
        )

    def emit_table_chunk_out(l, ci, table_tile):
        """DMA chunk ci of layer l's table into its AG bounce buffer(s)."""
        b0, b1 = ci * G, (ci + 1) * G
        for side, lo_b, hi_b in ((0, b0, min(b1, HB)), (1, max(b0, HB), b1)):
            if lo_b >= hi_b:
                continue
            cl = lo_b - (0 if side == 0 else HB)
            n = hi_b - lo_b
            nc.sync.dma_start(
                ags[l][side][:, cl * 128:(cl + n) * 128],
                table_tile[:, lo_b * 128:hi_b * 128],
            )

    def emit_gather(l, ci, side):
        if side == 0:
            msg = msgp.tile([128, G * t_lo, F], FP16, tag="mlo", bufs=LA + 2)
            if "gather" not in _SKIP:
                nc.gpsimd.dma_gather(
                    msg[:], tabs[l][0][:],
                    il_sb[:, ci * G * t_lo * 8:(ci + 1) * G * t_lo * 8],
                    nlo, nlo, F, single_packet=_SP,
                    queue_num=gk[0] % 4,
                )
        else:
            msg = msgp.tile([128, G * t_hi, F], FP16, tag="mhi", bufs=LH + 2)
            if "gather" not in _SKIP:
                nc.gpsimd.dma_gather(
                    msg[:], tabs[l][1][:],
                    ih_sb[:, ci * G * t_hi * 8:(ci + 1) * G * t_hi * 8],
                    nhi, nhi, F, single_packet=_SP,
                    queue_num=gk[0] % 4,
                )
        gk[0] += 1
        msgs[(side, ci)] = msg

    # ---- layer-0 table is precomputed host-side (dis * x, replicated
    # in DRAM as tab0_lo/tab0_hi inputs); just load the own shard for
    # the self-loop matmuls and start gathering immediately.
    table0 = tabp.tile([128, P_PAD], FP16, tag="table")
    nc.sync.dma_start(table0[:], t0own_d[:])
    for k in range(LA):
        emit_gather(0, k, 0)
    for k in range(LH):
        emit_gather(0, k, 1)

    # ---- 3 GCN layers
    table_cur = table0
    for l in range(3):
        fo = F if l < 2 else FOUT
        table_next = (
            tabp.tile([128, P_PAD], FP16, tag="table", name=f"table{l+1}")
            if l < 2 else None
        )
        for ci in range(NCHUNK):
            if ci + LA < NCHUNK:
                emit_gather(l, ci + LA, 0)
            if ci + LH < NCHUNK:
                emit_gather(l, ci + LH, 1)
            msg_lo = msgs.pop((0, ci))
            msg_hi = msgs.pop((1, ci))
            oh = ohp.tile([128, G * t_tot, 128], mybir.dt.float8e4, tag="oh")
            if "oh" not in _SKIP:
                if _OHGEN:
                    sl = slot_sb[:, ci * G * t_tot:(ci + 1) * G * t_tot]
                    sl_b = bass.AP(
                        sl.tensor, sl.offset,
                        list(sl.ap[:-1]) + [list(sl.ap[-1]), [0, 128]],
                    )
                    nc.vector.tensor_tensor(
                        out=oh[:], in0=iota_sb[:], in1=sl_b,
                        op=mybir.AluOpType.is_equal,
                    )
                else:
                    nc.scalar.dma_start(
                        oh[:], oh_d[:, ci * G * t_tot:(ci + 1) * G * t_tot, :]
                    )
            for bi in range(G):
                b = ci * G + bi
                agg_ps = ps.tile([128, 128], mybir.dt.float32, tag="agg", space="PSUM")
                # transposed segsum: aggT[feat, dst] += msg.T @ onehot
                if "mm" not in _SKIP:
                    for j in range(t_tot):
                        lhs = (
                            msg_lo[:, bi * t_lo + j, :] if j < t_lo
                            else msg_hi[:, bi * t_hi + (j - t_lo), :]
                        )
                        nc.tensor.matmul(
                            agg_ps[:], lhsT=lhs, rhs=oh[:, bi * t_tot + j, :],
                            start=(j == 0), stop=False,
                        )
                # self-loop: += table_blk.T @ I
                nc.tensor.matmul(
                    agg_ps[:], lhsT=table_cur[:, b * 128:(b + 1) * 128],
                    rhs=ident16[:],
                    start=("mm" in _SKIP), stop=True,
                )
                # PSUM readout on the ACT engine (keeps DVE free)
                aggT = sb.tile([128, 128], FP16, tag="aggT")
                nc.scalar.copy(aggT[:], agg_ps[:])
                h_ps = ps.tile([128, F], mybir.dt.float32, tag="hps", space="PSUM")
                nc.tensor.matmul(
                    h_ps[:, :fo], lhsT=aggT[:], rhs=w_sb[l][:],
                    start=True, stop=True,
                )
                # t1 = dis * h  (ACT reads PSUM, frees the bank early)
                t1 = sb.tile([128, F], mybir.dt.float32, tag="t1")
                nc.scalar.mul(t1[:, :fo], h_ps[:, :fo], dis_sb[:, b:b + 1])
                if l < 2:
                    # tmp2 = t1 + bias; table_next = dis * relu(tmp2)
                    tmp2 = sb.tile([128, F], mybir.dt.float32, tag="tmp2")
                    nc.vector.tensor_tensor(
                        out=tmp2[:], in0=t1[:], in1=bt_sb[l][:],
                        op=mybir.AluOpType.add,
                    )
                    nc.vector.tensor_scalar(
                        out=table_next[:, b * 128:(b + 1) * 128],
                        in0=tmp2[:],
                        scalar1=0.0, scalar2=dis_sb[:, b:b + 1],
                        op0=mybir.AluOpType.max, op1=mybir.AluOpType.mult,
                    )
                else:
                    ob = sb.tile([128, FOUT], mybir.dt.float32, tag="ob")
                    nc.vector.tensor_tensor(
                        out=ob[:], in0=t1[:, :FOUT], in1=bt_sb[2][:],
                        op=mybir.AluOpType.add,
                    )
                    nc.sync.dma_start(out_d[b * 128:(b + 1) * 128, :], ob[:])
            if l < 2:
                emit_table_chunk_out(l + 1, ci, table_next)
                # lo AG for the next layer: data ready after chunk HCHUNK-1;
                # trigger emitted 2 chunks later so queued gathers are not
                # head-of-line blocked behind its wait.
                if ci == CI_LO_READY + 3:
                    emit_ag(l + 1, 0)
        if l < 2:
            # next-layer prologue: lo gathers (tab_lo already gathered),
            # then the hi AG trigger, then the leading hi gathers.
            for k in range(LA):
                emit_gather(l + 1, k, 0)
            emit_ag(l + 1, 1)
            for k in range(LH):
                emit_gather(l + 1, k, 1)
            table_cur = table_next


def _timed_run(nc, in_maps, iters=5):
    """Mirror run_bass_via_pjrt's multi-core path, but keep inputs device-
    resident and time repeated executions (returns results, best_ns)."""
    import time
    import jax

    sharded, concat_in, make_zeros, unpack = _make_sharded(nc, in_maps)
    # Differential pipelined timing: dispatch n asynchronously, block once.
    # per-exec = (T(n_long) - T(n_short)) / (n_long - n_short) cancels the
    # RPC floor. Repeat pairs and take the min slope.
    n_short, n_long, pairs = 4, 16, max(2, iters)
    n_total = 1 + pairs * (n_short + n_long)
    zero_sets = [make_zeros() for _ in range(n_total)]
    out_arrs = jax.block_until_ready(sharded(*concat_in, *zero_sets[0]))

    def pipe(k0, n):
        t0 = time.perf_counter()
        rs = [sharded(*concat_in, *zero_sets[k0 + k]) for k in range(n)]
        last = jax.block_until_ready(rs[-1])
        dt = time.perf_counter() - t0
        for r in rs[:-1]:
            del r
        return dt, last

    times = []
    slopes = []
    k0 = 1
    for _ in range(pairs):
        t_s, _ = pipe(k0, n_short)
        k0 += n_short
        t_l, out_arrs = pipe(k0, n_long)
        k0 += n_long
        slopes.append((t_l - t_s) / (n_long - n_short))
        times.extend([t_s, t_l])
    pos = [s for s in slopes if s > 0] or slopes
    per_exec = min(pos)
    best_ns = int(per_exec * 1e9)
    kernel._last_slopes = [int(s * 1e9) for s in slopes]
    results = unpack(out_arrs)
    return results, best_ns, times


def _make_sharded(nc, in_maps):
    """Build the sharded jit fn + device-resident inputs (shared helper)."""
    import jax
    from jax.sharding import Mesh, PartitionSpec, NamedSharding
    from jax.experimental.shard_map import shard_map
    import concourse.mybir as mb
    from concourse.bass2jax import (
        _bass_exec_p, partition_id_tensor, install_neuronx_cc_hook,
    )

    install_neuronx_cc_hook()
    n_cores = len(in_maps)
    partition_name = nc.partition_id_tensor.name if nc.partition_id_tensor else None
    in_names, out_names, out_avals, zero_outs = [], [], [], []
    for alloc in nc.m.functions[0].allocations:
        if not isinstance(alloc, mb.MemoryLocationSet):
            continue
        name = alloc.memorylocations[0].name
        if alloc.kind == "ExternalInput":
            if name != partition_name:
                in_names.append(name)
        elif alloc.kind == "ExternalOutput":
            out_names.append(name)
            shape = tuple(alloc.tensor_shape)
            dtype = mb.dt.np(alloc.dtype)
            out_avals.append(jax.core.ShapedArray(shape, dtype))
            zero_outs.append(np.zeros(shape, dtype))
    n_params = len(in_names)
    n_outs = len(out_avals)
    in_names.extend(out_names)
    if partition_name is not None:
        in_names.append(partition_name)
    donate = tuple(range(n_params, n_params + n_outs))

    def _body(*args):
        operands = list(args)
        if partition_name is not None:
            operands.append(partition_id_tensor())
        return tuple(_bass_exec_p.bind(
            *operands,
            out_avals=tuple(out_avals), in_names=tuple(in_names),
            out_names=tuple(out_names), lowering_input_output_aliases=(),
            sim_require_finite=True, sim_require_nnan=True, nc=nc,
        ))

    devices = jax.devices()[:n_cores]
    mesh = Mesh(np.asarray(devices), ("core",))
    spec = NamedSharding(mesh, PartitionSpec("core"))
    sharded = jax.jit(
        shard_map(_body, mesh=mesh,
                  in_specs=(PartitionSpec("core"),) * (n_params + n_outs),
                  out_specs=(PartitionSpec("core"),) * n_outs,
                  check_rep=False),
        donate_argnums=donate, keep_unused=True,
    )
    concat_in = [
        jax.device_put(
            np.concatenate([np.asarray(in_maps[c][in_names[i]]) for c in range(n_cores)], axis=0),
            spec,
        )
        for i in range(n_params)
    ]

    def make_zeros():
        return [jax.device_put(np.zeros((n_cores * z.shape[0], *z.shape[1:]), z.dtype), spec)
                for z in zero_outs]

    def unpack(out_arrs):
        return [
            {name: np.asarray(out_arrs[i]).reshape(n_cores, *out_avals[i].shape)[c]
             for i, name in enumerate(out_names)}
            for c in range(n_cores)
        ]

    return sharded, concat_in, make_zeros, unpack


def _prepare(x, edge_index, W0, b0, W1, b1, W2, b2):
    """Preprocess + build program + per-core input maps.

    Returns (nc, in_maps, unshard) where unshard(results) -> full output.
    """
    x = np.asarray(x)
    edge_index = np.asarray(edge_index)
    per_core, pp, dis, node_block, node_slot, t_lo, t_hi = _preprocess(edge_index)

    nc = _build_program(t_lo, t_hi)

    w0 = np.ascontiguousarray(np.asarray(W0, np.float16))
    w1 = np.ascontiguousarray(np.asarray(W1, np.float16))
    w2 = np.ascontiguousarray(np.asarray(W2, np.float16))
    bt0 = np.tile(np.asarray(b0, np.float32)[None, :], (128, 1))
    bt1 = np.tile(np.asarray(b1, np.float32)[None, :], (128, 1))
    bt2 = np.tile(np.asarray(b2, np.float32)[None, :], (128, 1))

    # host-precomputed layer-0 table (dis * x), in the AllGather layout
    pre0 = (dis[:, None] * np.asarray(x, np.float32)).astype(np.float16)
    tab0_lo = np.zeros((HALF, F), np.float16)
    tab0_hi = np.zeros((HALF, F), np.float16)
    lo_m = pp < HALF
    tab0_lo[pp[lo_m]] = pre0[lo_m]
    tab0_hi[pp[~lo_m] - HALF] = pre0[~lo_m]
    tab0_lo = np.ascontiguousarray(tab0_lo)
    tab0_hi = np.ascontiguousarray(tab0_hi)

    in_maps = []
    for c in range(NC):
        vs = np.arange(c * P_OWN, (c + 1) * P_OWN)
        rows = node_block[vs] * 128 + node_slot[vs]  # padded local row
        t_nm = np.zeros((P_PAD, F), np.float16)
        t_nm[rows] = pre0[vs]
        # p-major: [128 slot, BLOCKS*128] where col b*128+f = tab[node(b,p), f]
        t0own = np.ascontiguousarray(
            t_nm.reshape(BLOCKS, 128, F).transpose(1, 0, 2).reshape(128, P_PAD)
        )
        dis_b = np.zeros((128, BLOCKS), np.float32)
        dis_b[node_slot[vs], node_block[vs]] = dis[vs]
        d = per_core[c]
        im = dict(
            tab0_lo=tab0_lo, tab0_hi=tab0_hi, t0own=t0own, dis_d=dis_b,
            idx_lo=np.ascontiguousarray(d["idx_lo"]),
            idx_hi=np.ascontiguousarray(d["idx_hi"]),
            w0=w0, w1=w1, w2=w2, bt0=bt0, bt1=bt1, bt2=bt2,
        )
        if _OHGEN:
            im["slots"] = np.ascontiguousarray(d["slots"])
        else:
            im["onehot"] = np.ascontiguousarray(d["onehot"])
        in_maps.append(im)

    def unshard(results):
        out = np.empty((N, FOUT), np.float32)
        for c in range(NC):
            vs = np.arange(c * P_OWN, (c + 1) * P_OWN)
            rows = node_block[vs] * 128 + node_slot[vs]
            out[vs] = results[c]["out"][rows]
        return out

    return nc, in_maps, unshard


def kernel(x, edge_index, W0, b0, W1, b1, W2, b2, _trace=False, _bench_iters=0):
    nc, in_maps, unshard = _prepare(x, edge_index, W0, b0, W1, b1, W2, b2)

    if _bench_iters:
        results, best_ns, times = _timed_run(nc, in_maps, iters=_bench_iters)
        kernel._last_time_ns = best_ns
        kernel._last_times = times
    else:
        res = run_bass_kernel_spmd(nc, in_maps, core_ids=list(range(NC)), trace=_trace)
        results = res.results
        if _trace:
            kernel._last_result = res

    return unshard(results)
